# Initial kernel scaffold
#
"""Trainium2 Bass kernel for nn_GAT_GCN (GAT conv + GCN conv + pooling + MLP tail).

Strategy (8 NeuronCores, SPMD, full inputs in / full output out):
  - Nodes are sharded by graph: core c owns graphs [16c, 16c+16). Within a core,
    nodes are laid out in graph-aligned "slots" (MAXG slots per graph) so pooling
    reduces over fixed-size slot ranges (uniform program across cores).
  - Edges are sharded by destination node. Per 128-slot destination block, edges
    are processed in subtiles of 128; a host-built one-hot mask turns the
    per-destination segment-sum into a TensorEngine matmul.
  - GAT: gather per-edge payload [x[src] | a_src[src]] (256 B rows, bf16) from a
    replicated table; attention softmax folded into the mask matmul (exp(e)
    weights in the rhs, normalization by the per-destination sum afterwards).
    The head-blocked weight multiply (y @ W_h) runs as a separate pass using
    DMA-transpose loads.
  - GCN: z rows (scaled by dinv[src] at production time) are gathered directly
    as the matmul rhs; dinv[dst] is applied when copying out of PSUM. The dense
    [1140x1140] multiply runs as a transposed pass producing zfin^T, which
    feeds max/mean pooling via free-dim reductions.
  - Cross-core exchange: AllGather of the small payload table and of z
    (bf16), interleaved across the two branches to hide latency.
  - The tiny MLP tail is computed replicated on every core.

Host-side preprocessing is restricted to index manipulation (sorting/bucketing
edges, one-hot masks, padding) and parameter repacking (padding / bf16 casts /
tiny reshapes of weights) -- all data-dependent float compute runs on device.
"""

import math

import numpy as np
import ml_dtypes

import concourse.bacc as bacc
import concourse.bass as bass
import concourse.tile as tile
from concourse import mybir, library_config
from concourse.bass_utils import run_bass_kernel_spmd
from concourse.tile_rust import add_dep_helper

# ---------------------------------------------------------------- constants
N = 20000
E = 160000
G = 128
F = 114
H = 10
HF = 1140          # F * H
NC = 8
GPC = G // NC      # graphs per core
P = 128

bf16 = mybir.dt.bfloat16
f32 = mybir.dt.float32
i16 = mybir.dt.int16

BF = ml_dtypes.bfloat16

_PROG_CACHE: dict = {}


# ---------------------------------------------------------------- host utils
def _wrap_idx(idx: np.ndarray) -> np.ndarray:
    """int16 index list (len % 16 == 0) -> [128, len/16] wrapped layout."""
    n = idx.shape[0]
    assert n % 16 == 0
    return np.tile(idx.reshape(-1, 16).T, (8, 1)).astype(np.int16)


def _prep_branch(x, ei, batch, gatW, att_src, att_dst):
    """Host preprocessing for one branch. Returns (static, percore, shared)."""
    x = np.asarray(x, dtype=np.float32)
    ei = np.asarray(ei).astype(np.int64)
    batch = np.asarray(batch).astype(np.int64)
    gatW = np.asarray(gatW, dtype=np.float32)
    att_src = np.asarray(att_src, dtype=np.float32)
    att_dst = np.asarray(att_dst, dtype=np.float32)

    cnt = np.bincount(batch, minlength=G)
    MAXG = int(cnt.max())
    SS = ((16 * MAXG + 127) // 128) * 128       # slots per core shard
    NBLK = SS // 128
    assert NC * SS < 32768, "row ids must fit int16"

    gstart = np.zeros(G + 1, np.int64)
    np.cumsum(cnt, out=gstart[1:])
    nodes = np.arange(N)
    rank = nodes - gstart[batch]
    slot_of_node = (batch % GPC) * MAXG + rank          # [N] in [0, 16*MAXG)
    core_of_node = batch // GPC                          # [N]
    row_of_node = core_of_node * SS + slot_of_node       # [N] global table row

    src = np.concatenate([ei[0], nodes])
    dst = np.concatenate([ei[1], nodes])
    NE = src.shape[0]

    core_e = core_of_node[dst]
    slot_e = slot_of_node[dst]
    blk_e = slot_e // 128
    dloc_e = slot_e % 128

    # bucket edges per (core, block)
    order = np.lexsort((blk_e, core_e))
    src_s, core_s, blk_s, dloc_s = src[order], core_e[order], blk_e[order], dloc_e[order]
    key = core_s * NBLK + blk_s
    counts = np.bincount(key, minlength=NC * NBLK)
    starts = np.zeros(NC * NBLK + 1, np.int64)
    np.cumsum(counts, out=starts[1:])
    NSUB = int(max(1, -(-counts.max() // 128)))
    CAP = NSUB * 128

    percore = []
    for c in range(NC):
        isrc = np.zeros((NBLK, CAP), np.int64)
        mask = np.zeros((NBLK, 128, NSUB, 128), np.float32)   # [e, s, dloc]
        for b in range(NBLK):
            k = c * NBLK + b
            cnt_b = counts[k]
            sl = slice(starts[k], starts[k + 1])
            isrc[b, :cnt_b] = row_of_node[src_s[sl]]
            pos = np.arange(cnt_b)
            mask[b, pos % 128, pos // 128, dloc_s[sl]] = 1.0
        maskT = np.ascontiguousarray(mask.transpose(0, 3, 2, 1))  # [d, s, e]
        isrc_w = np.stack([_wrap_idx(isrc[b].astype(np.int16)) for b in range(NBLK)])

        # per-slot metadata for this core
        slots = np.arange(SS)
        g_local = np.minimum(slots // MAXG, GPC - 1)
        r_local = slots - g_local * MAXG
        g_global = c * GPC + g_local
        validity = (slots < 16 * MAXG) & (r_local < cnt[g_global])
        node_of_slot = np.zeros(SS, np.int64)
        real = validity.nonzero()[0]
        node_of_slot[real] = gstart[g_global[real]] + r_local[real]
        xidx = np.stack([_wrap_idx(node_of_slot[b * 128:(b + 1) * 128].astype(np.int16))
                         for b in range(NBLK)])

        deg = np.bincount(dst, minlength=N).astype(np.float64)
        dinv_node = 1.0 / np.sqrt(np.maximum(deg, 1.0))
        dinv_slot = np.ones(SS, np.float32)
        dinv_slot[real] = dinv_node[node_of_slot[real]]

        valid = validity.astype(np.float32)
        poison = np.where(validity, 0.0, -1e28).astype(np.float32)
        rcnt = (1.0 / np.maximum(cnt[c * GPC:(c + 1) * GPC], 1.0)).astype(np.float32)

        percore.append({
            "isrc": isrc_w.astype(np.int16),                        # [NBLK,128,NSUB*8]
            "mask": mask.astype(BF),                                # [NBLK,128,NSUB,128]
            "maskT": maskT.astype(BF),
            "xidx": xidx.astype(np.int16),                          # [NBLK,128,8]
            "dinv": dinv_slot.reshape(NBLK, 128, 1),                # f32
            "valid": valid.reshape(NBLK, 128, 1),
            "inval": (1.0 - valid).reshape(NBLK, 128, 1),
            "poison": np.broadcast_to(poison.astype(BF), (128, SS)).copy(),
            "rcnt": np.broadcast_to(rcnt, (128, GPC)).copy(),
        })

    # shared (weights / x table)
    xpad = np.zeros((N, 128), np.float32)
    xpad[:, :F] = x
    gatW3 = gatW.reshape(F, H, F)
    Ws = np.einsum("khf,hf->kh", gatW3, att_src)
    Wd = np.einsum("khf,hf->kh", gatW3, att_dst)
    Wsd = np.zeros((128, 2 * H), np.float32)
    Wsd[:F, :H] = Ws
    Wsd[:F, H:] = Wd

    shared = {
        "xpad": xpad.astype(BF),              # [N,128] bf16
        "Wsd": Wsd.astype(BF),                # [128,20]
    }
    static = dict(MAXG=MAXG, SS=SS, NBLK=NBLK, NSUB=NSUB)
    return static, percore, shared


def _pack_branch_weights(gatW, gatb, gcnW, gcnb):
    gatW = np.asarray(gatW, np.float32)
    gatb = np.asarray(gatb, np.float32)
    gcnW = np.asarray(gcnW, np.float32)
    gcnb = np.asarray(gcnb, np.float32)
    gatWk = np.zeros((H, 128, F), np.float32)
    gatW3 = gatW.reshape(F, H, F)
    for h in range(H):
        gatWk[h, :F, :] = gatW3[:, h, :]
        gatWk[h, F, :] = gatb[h * F:(h + 1) * F]
    gcn_pad = np.zeros((1152, 1152), np.float32)
    gcn_pad[:HF, :HF] = gcnW
    gcn_pad[HF, :HF] = gcnb
    gcnWk = gcn_pad.reshape(9, 128, 1152)
    return gatWk.astype(BF), gcnWk.astype(BF)


def _pack_tail(inp):
    f = lambda k: np.asarray(inp[k], np.float32)
    tail = {}
    for p in ("p1", "p2"):
        W1 = np.zeros((2432, 1024), np.float32)
        fg1 = f(p + "_fcg1W")           # [2280, 1000]
        W1[0:HF, 0:1000] = fg1[0:HF]
        W1[1152:1152 + HF, 0:1000] = fg1[HF:2 * HF]
        k1 = np.concatenate([W1[:2304].reshape(18, 128, 1024),
                             np.zeros((1, 128, 1024), np.float32)], axis=0)
        k1[18, 0, 0:1000] = f(p + "_fcg1b")
        tail[p + "_fcg1Wk"] = k1.astype(np.float32)
        W2 = np.zeros((1024, 64), np.float32)
        W2[0:1000] = f(p + "_fcg2W")
        W2[1000] = f(p + "_fcg2b")
        tail[p + "_fcg2Wk"] = W2.reshape(8, 128, 64).astype(np.float32)
    Wx = np.zeros((1024, 128), np.float32)
    Wx[0:1000] = f("fcxtW")
    Wx[1000] = f("fcxtb")
    tail["fcxtWk"] = Wx.reshape(8, 128, 128).astype(np.float32)
    W1 = np.zeros((3, 128, 128), np.float32)
    W1[0] = f("fc1W")[0:128]
    W1[1] = f("fc1W")[128:256]
    W1[2, 0] = f("fc1b")
    tail["fc1Wk"] = W1.astype(np.float32)
    W2 = np.zeros((2, 128, 32), np.float32)
    W2[0] = f("fc2W")
    W2[1, 0] = f("fc2b")
    tail["fc2Wk"] = W2.astype(np.float32)
    Wo = np.zeros((128, 1), np.float32)
    Wo[0:32, 0] = f("outW")[:, 0]
    Wo[32, 0] = float(np.asarray(inp["outb"]).reshape(-1)[0])
    tail["outWk"] = Wo.astype(np.float32)
    tail["identity"] = np.eye(128, dtype=np.float32)
    tail["target"] = f("target")
    return tail


# ---------------------------------------------------------------- device build
GATHER_MAX = 1024  # dma_gather breaks above 1024 indices per call (HW-probed)


def _gather_chunked(nc, pools, out_tile, table_ap, ii, cap, elem, deps=()):
    """Emit dma_gather in <=1024-index chunks. out_tile is [128, cap/128, elem]."""
    insts = []
    for i0 in range(0, cap, GATHER_MAX):
        i1 = min(cap, i0 + GATHER_MAX)
        n = i1 - i0
        g = nc.gpsimd.dma_gather(
            out_tile[:, i0 // 128:i1 // 128, :], table_ap,
            ii[:, i0 // 16:i1 // 16], n, n, elem)
        add_dep_helper(g.ins, pools["lib"].ins, reason="gather after lib")
        for dd in deps:
            add_dep_helper(g.ins, dd.ins, reason="gather dep")
        insts.append(g)
    return insts


def _build_branch(nc, tc, ctx, pools, pfx, st, deps):
    """Emit phases T-build, AG_T, GAT-agg, GAT-W, AG_z for one branch.

    Returns a dict with handles needed by the GCN phases.
    """
    MAXG, SS, NBLK, NSUB = st["MAXG"], st["SS"], st["NBLK"], st["NSUB"]
    CAP = NSUB * 128
    d = nc.dram_tensor
    # inputs
    xpad = d(pfx + "xpad", [N, 128], bf16, kind="ExternalInput")
    xidx = d(pfx + "xidx", [NBLK, 128, 8], i16, kind="ExternalInput")
    isrc = d(pfx + "isrc", [NBLK, 128, NSUB * 8], i16, kind="ExternalInput")
    maskD = d(pfx + "mask", [NBLK, 128, NSUB, 128], bf16, kind="ExternalInput")
    maskTD = d(pfx + "maskT", [NBLK, 128, NSUB, 128], bf16, kind="ExternalInput")
    dinvD = d(pfx + "dinv", [NBLK, 128, 1], f32, kind="ExternalInput")
    invalD = d(pfx + "inval", [NBLK, 128, 1], f32, kind="ExternalInput")
    validD = d(pfx + "valid", [NBLK, 128, 1], f32, kind="ExternalInput")
    WsdD = d(pfx + "Wsd", [128, 2 * H], bf16, kind="ExternalInput")
    gatWkD = d(pfx + "gatWk", [H, 128, F], bf16, kind="ExternalInput")
    # internals
    T_loc = d(pfx + "T_loc", [SS, 256], bf16)
    T_glob = d(pfx + "T_glob", [NC * SS, 256], bf16, addr_space="Shared")
    y_dram = d(pfx + "y", [SS, 1290], bf16)
    z_loc = d(pfx + "z_loc", [SS, 1152], bf16)
    z_glob = d(pfx + "z_glob", [NC * SS, 1152], bf16, addr_space="Shared")

    sb, ps = pools["sb"], pools["ps"]
    rg = [list(range(NC))]

    # weights resident for this branch section
    Wsd = sb.tile([128, 2 * H], bf16, tag="wsd")
    nc.sync.dma_start(Wsd[:], WsdD[:])
    gatWk = sb.tile([128, H, F], bf16, tag="gatwk")
    nc.sync.dma_start(gatWk[:], gatWkD.ap().rearrange("h k n -> k h n"))

    # ---------------- Phase 1: T-build ----------------
    t_writes = []
    for b in range(NBLK):
        xi = sb.tile([128, 8], i16, tag="xi")
        nc.sync.dma_start(xi[:], xidx[b])
        xg = sb.tile([128, 1, 128], bf16, tag="xg")
        g1 = nc.gpsimd.dma_gather(xg[:], xpad[:], xi[:], 128, 128, 128)
        add_dep_helper(g1.ins, pools["lib"].ins, reason="gather after lib")
        xgT = sb.tile([128, 1, 128], bf16, tag="xgT")
        g2 = nc.gpsimd.dma_gather(xgT[:], xpad[:], xi[:], 128, 128, 128, transpose=True)
        add_dep_helper(g2.ins, pools["lib"].ins, reason="gather after lib")
        aps = ps.tile([128, 2 * H], f32, tag="ps_small")
        nc.tensor.matmul(aps[:], xgT[:, 0, :], Wsd[:], start=True, stop=True)
        Tt = sb.tile([128, 256], bf16, tag="Tt")
        nc.vector.tensor_copy(Tt[:, 0:F], xg[:, 0, 0:F])
        nc.vector.tensor_copy(Tt[:, F:F + 2 * H], aps[:])
        nc.vector.memset(Tt[:, F + 2 * H:256], 0.0)
        w = nc.sync.dma_start(T_loc[b * 128:(b + 1) * 128, :], Tt[:])
        t_writes.append(w)
    ag_t = nc.gpsimd.collective_compute(
        "AllGather", mybir.AluOpType.bypass, replica_groups=rg,
        ins=[T_loc[:]], outs=[T_glob[:]])
    for w in t_writes:
        add_dep_helper(ag_t.ins, w.ins, reason="AG_T after T writes")

    # ---------------- Phase 2: GAT aggregation ----------------
    y_writes = []
    for b in range(NBLK):
        ii = sb.tile([128, NSUB * 8], i16, tag="ii")
        nc.sync.dma_start(ii[:], isrc[b])
        S = sb.tile([128, NSUB, 256], bf16, tag="S")
        _gather_chunked(nc, pools, S, T_glob[:], ii, CAP, 256, deps=(ag_t,))
        Mt = sb.tile([128, NSUB, 128], bf16, tag="Mt")
        nc.sync.dma_start(Mt[:], maskD[b])
        MtT = sb.tile([128, NSUB, 128], bf16, tag="MtT")
        nc.sync.dma_start(MtT[:], maskTD[b])
        adb = sb.tile([128, 2 * H], bf16, tag="adb")
        r = nc.sync.dma_start(adb[:], T_loc[b * 128:(b + 1) * 128, F:F + 2 * H])
        add_dep_helper(r.ins, t_writes[b].ins, reason="adb after T write")
        inval = sb.tile([128, 1], f32, tag="col")
        nc.sync.dma_start(inval[:], invalD[b])
        dinv = sb.tile([128, 1], f32, tag="col2")
        nc.sync.dma_start(dinv[:], dinvD[b])

        lg = sb.tile([128, NSUB, H], f32, tag="lg")
        for s in range(NSUB):
            ad_ps = ps.tile([128, H], f32, tag="ps_small")
            nc.tensor.matmul(ad_ps[:], MtT[:, s, :], adb[:, H:2 * H],
                             start=True, stop=True)
            nc.vector.tensor_tensor(out=lg[:, s, :], in0=S[:, s, F:F + H],
                                    in1=ad_ps[:], op=mybir.AluOpType.add)
        l3 = sb.tile([128, NSUB, H], f32, tag="l3")
        nc.vector.scalar_tensor_tensor(out=l3[:], in0=lg[:], scalar=0.2, in1=lg[:],
                                       op0=mybir.AluOpType.mult,
                                       op1=mybir.AluOpType.max)
        exb = sb.tile([128, NSUB, H], bf16, tag="exb")
        nc.scalar.activation(exb[:], l3[:], mybir.ActivationFunctionType.Exp)

        R = sb.tile([128, NSUB, 1150], bf16, tag="R")
        nc.vector.tensor_tensor(
            out=R[:, :, 0:HF].rearrange("p s (h f) -> p s h f", h=H),
            in0=S[:, :, 0:F].unsqueeze(2).broadcast_to([128, NSUB, H, F]),
            in1=exb[:].unsqueeze(3).broadcast_to([128, NSUB, H, F]),
            op=mybir.AluOpType.mult)
        nc.vector.tensor_copy(R[:, :, HF:1150], exb[:])

        y_ps = ps.tile([128, 1150], f32, tag="ps_big")
        for s in range(NSUB):
            for c0, c1 in ((0, 512), (512, 1024), (1024, 1150)):
                nc.tensor.matmul(y_ps[:, c0:c1], Mt[:, s, :], R[:, s, c0:c1],
                                 start=(s == 0), stop=(s == NSUB - 1))

        den = sb.tile([128, H], f32, tag="den")
        nc.vector.tensor_scalar(out=den[:], in0=y_ps[:, HF:1150], scalar1=inval[:],
                                scalar2=None, op0=mybir.AluOpType.add)
        rden = sb.tile([128, H], f32, tag="rden")
        nc.vector.reciprocal(rden[:], den[:])
        rden2 = sb.tile([128, H], bf16, tag="rden2")
        nc.vector.tensor_scalar(out=rden2[:], in0=rden[:], scalar1=dinv[:],
                                scalar2=None, op0=mybir.AluOpType.mult)

        yt = sb.tile([128, 1290], bf16, tag="yt")
        ytv = yt[:, 0:1280].rearrange("p (h c) -> p h c", h=H)
        ypv = y_ps[:, 0:HF].rearrange("p (h f) -> p h f", h=H)
        nc.vector.tensor_copy(ytv[:, 0:5, 0:F], ypv[:, 0:5, :])
        nc.scalar.copy(ytv[:, 5:H, 0:F], ypv[:, 5:H, :])
        nc.vector.tensor_copy(ytv[:, :, F:F + 1], den[:].unsqueeze(2))
        nc.vector.memset(ytv[:, :, F + 1:128], 0.0)
        nc.vector.tensor_copy(yt[:, 1280:1290], rden2[:])
        w = nc.sync.dma_start(y_dram[b * 128:(b + 1) * 128, :], yt[:])
        y_writes.append(w)

    # ---------------- Phase 3: GAT W-pass (z production) ----------------
    z_writes = []
    for t in range(NBLK):
        rdn = sb.tile([128, H], bf16, tag="rdn")
        r = nc.sync.dma_start(rdn[:], y_dram[t * 128:(t + 1) * 128, 1280:1290])
        add_dep_helper(r.ins, y_writes[t].ins, reason="rden after y write")
        zp = ps.tile([128, 1280], f32, tag="ps_big")
        for h in range(H):
            yT = sb.tile([128, 128], bf16, tag="yT")
            ld = nc.sync.dma_start_transpose(
                out=yT[:], in_=y_dram[t * 128:(t + 1) * 128, h * 128:(h + 1) * 128])
            add_dep_helper(ld.ins, y_writes[t].ins, reason="yT after y write")
            bank = (h * 128) // 512
            first = (h % 4 == 0)
            last = (h % 4 == 3) or (h == H - 1)
            nc.tensor.matmul(zp[:, h * 128:h * 128 + F], yT[:], gatWk[:, h, :],
                             start=first, stop=last)
        zt = sb.tile([128, 1152], bf16, tag="zt")
        zpre = sb.tile([128, HF], f32, tag="zpre")
        nc.vector.tensor_tensor(
            out=zpre[:].rearrange("p (h f) -> p h f", h=H),
            in0=zp[:].rearrange("p (h c) -> p h c", h=H)[:, :, 0:F],
            in1=rdn[:].unsqueeze(2).broadcast_to([128, H, F]),
            op=mybir.AluOpType.mult)
        nc.vector.scalar_tensor_tensor(out=zt[:, 0:HF], in0=zpre[:], scalar=0.01,
                                       in1=zpre[:], op0=mybir.AluOpType.mult,
                                       op1=mybir.AluOpType.max)
        nc.vector.memset(zt[:, HF:1152], 0.0)
        w = nc.sync.dma_start(z_loc[t * 128:(t + 1) * 128, :], zt[:])
        z_writes.append(w)
    ag_z = nc.gpsimd.collective_compute(
        "AllGather", mybir.AluOpType.bypass, replica_groups=rg,
        ins=[z_loc[:]], outs=[z_glob[:]])
    for w in z_writes:
        add_dep_helper(ag_z.ins, w.ins, reason="AG_z after z writes")

    return dict(st=st, isrc=isrc, maskD=maskD, dinvD=dinvD, validD=validD,
                z_glob=z_glob, ag_z=ag_z, pfx=pfx)


def _build_gcn(nc, tc, ctx, pools, br, pool_loc, pool_col0, plw):
    """GCN aggregation + W-pass + pooling for one branch."""
    st = br["st"]
    MAXG, SS, NBLK, NSUB = st["MAXG"], st["SS"], st["NBLK"], st["NSUB"]
    CAP = NSUB * 128
    pfx = br["pfx"]
    d = nc.dram_tensor
    gcnWkD = d(pfx + "gcnWk", [9, 128, 1152], bf16, kind="ExternalInput")
    poisonD = d(pfx + "poison", [128, SS], bf16, kind="ExternalInput")
    rcntD = d(pfx + "rcnt", [128, GPC], f32, kind="ExternalInput")
    y2_dram = d(pfx + "y2", [SS, 1152], bf16)

    sb, ps = pools["sb"], pools["ps"]

    # ---------------- Phase 4: GCN aggregation ----------------
    y2_writes = []
    for b in range(NBLK):
        ii = sb.tile([128, NSUB * 8], i16, tag="ii")
        nc.sync.dma_start(ii[:], br["isrc"][b])
        Z = sb.tile([128, NSUB, 1152], bf16, tag="Z")
        _gather_chunked(nc, pools, Z, br["z_glob"][:], ii, CAP, 1152,
                        deps=(br["ag_z"],))
        Mt = sb.tile([128, NSUB, 128], bf16, tag="Mt")
        nc.sync.dma_start(Mt[:], br["maskD"][b])
        dinv = sb.tile([128, 1], f32, tag="col2")
        nc.sync.dma_start(dinv[:], br["dinvD"][b])
        valid = sb.tile([128, 1], f32, tag="col")
        nc.sync.dma_start(valid[:], br["validD"][b])

        y2_ps = ps.tile([128, HF], f32, tag="ps_big")
        for s in range(NSUB):
            for c0, c1 in ((0, 512), (512, 1024), (1024, HF)):
                nc.tensor.matmul(y2_ps[:, c0:c1], Mt[:, s, :], Z[:, s, c0:c1],
                                 start=(s == 0), stop=(s == NSUB - 1))
        y2t = sb.tile([128, 1152], bf16, tag="zt")
        nc.vector.tensor_scalar(out=y2t[:, 0:512], in0=y2_ps[:, 0:512],
                                scalar1=dinv[:], scalar2=None,
                                op0=mybir.AluOpType.mult)
        nc.scalar.activation(y2t[:, 512:HF], y2_ps[:, 512:HF],
                             mybir.ActivationFunctionType.Copy, scale=dinv[:])
        nc.vector.tensor_copy(y2t[:, HF:HF + 1], valid[:])
        nc.vector.memset(y2t[:, HF + 1:1152], 0.0)
        w = nc.sync.dma_start(y2_dram[b * 128:(b + 1) * 128, :], y2t[:])
        y2_writes.append(w)

    # ---------------- Phase 5: GCN W-pass + pooling ----------------
    gcnWk = sb.tile([128, 9, 1152], bf16, tag="gcnwk", bufs=1)
    nc.sync.dma_start(gcnWk[:], gcnWkD.ap().rearrange("kb kr n -> kr kb n"))
    zfin = sb.tile([128, 9, SS], bf16, tag="zfin", bufs=1)
    groups = []
    r = 0
    while r < SS:
        groups.append((r, min(SS, r + 512)))
        r += 512
    for (r0, r1) in groups:
        gw = r1 - r0
        yTs = []
        for kb in range(9):
            y2T = sb.tile([128, 512], bf16, tag=f"y2T{kb}")
            ld = nc.sync.dma_start_transpose(
                out=y2T[:, 0:gw], in_=y2_dram[r0:r1, kb * 128:(kb + 1) * 128])
            for bb in range(r0 // 128, (r1 + 127) // 128):
                add_dep_helper(ld.ins, y2_writes[bb].ins, reason="y2T after y2 write")
            yTs.append(y2T)
        for nb in range(9):
            ct = ps.tile([128, 512], f32, tag="ps_small")
            for kb in range(9):
                nc.tensor.matmul(ct[:, 0:gw], gcnWk[:, kb, nb * 128:(nb + 1) * 128],
                                 yTs[kb][:, 0:gw], start=(kb == 0), stop=(kb == 8))
            nc.scalar.activation(zfin[:, nb, r0:r1], ct[:, 0:gw],
                                 mybir.ActivationFunctionType.Lrelu, alpha=0.01)

    # pooling
    poison = sb.tile([128, SS], bf16, tag="poison", bufs=1)
    nc.sync.dma_start(poison[:], poisonD[:])
    rcnt = sb.tile([128, GPC], f32, tag="rcnt")
    nc.sync.dma_start(rcnt[:], rcntD[:])
    mxT = sb.tile([128, 9, GPC], f32, tag="mxT")
    smT = sb.tile([128, 9, GPC], f32, tag="smT")
    for g in range(GPC):
        s0 = g * MAXG
        tmp = sb.tile([128, 9, MAXG], bf16, tag="ptmp")
        nc.vector.tensor_tensor(
            out=tmp[:], in0=zfin[:, :, s0:s0 + MAXG],
            in1=poison[:, s0:s0 + MAXG].unsqueeze(1).broadcast_to([128, 9, MAXG]),
            op=mybir.AluOpType.add)
        for ft in range(9):
            nc.vector.reduce_max(mxT[:, ft, g:g + 1], tmp[:, ft, :],
                                 axis=mybir.AxisListType.X)
            nc.vector.reduce_sum(smT[:, ft, g:g + 1], zfin[:, ft, s0:s0 + MAXG],
                                 axis=mybir.AxisListType.X)
    mnT = sb.tile([128, 9, GPC], f32, tag="mnT")
    nc.vector.tensor_tensor(out=mnT[:], in0=smT[:],
                            in1=rcnt[:].unsqueeze(1).broadcast_to([128, 9, GPC]),
                            op=mybir.AluOpType.mult)

    # stage into pool_loc[16, col0 : col0+2304]
    ident = pools["ident"]
    writes = []
    for which, statT in ((0, mxT), (1, mnT)):
        for ft in range(9):
            tp = ps.tile([GPC, 128], f32, tag="ps_small")
            nc.tensor.transpose(tp[:], statT[:, ft, :], ident[:])
            stg = sb.tile([GPC, 128], f32, tag="stg")
            nc.vector.tensor_copy(stg[:], tp[:])
            w = nc.sync.dma_start(
                pool_loc[:, pool_col0 + which * 1152 + ft * 128:
                         pool_col0 + which * 1152 + ft * 128 + 128], stg[:])
            add_dep_helper(w.ins, plw.ins, reason="stage after pool init")
            writes.append(w)
    return writes


def _build_tail(nc, tc, ctx, pools, pool_glob, ag_pool):
    d = nc.dram_tensor
    sb, ps = pools["sb"], pools["ps"]
    ident = pools["ident"]
    tgtD = d("target", [G, 1000], f32, kind="ExternalInput")
    fcxtWkD = d("fcxtWk", [8, 128, 128], f32, kind="ExternalInput")
    fc1WkD = d("fc1Wk", [3, 128, 128], f32, kind="ExternalInput")
    fc2WkD = d("fc2Wk", [2, 128, 32], f32, kind="ExternalInput")
    outWkD = d("outWk", [128, 1], f32, kind="ExternalInput")
    outD = d("out", [G, 1], f32, kind="ExternalOutput")

    def pe_T(src_ap, rows):
        tp = ps.tile([rows, 128], f32, tag="ps_small")
        nc.tensor.transpose(tp[:], src_ap, ident[:])
        return tp

    def mm_transposed(src_tile, nk, rhs_fn, psum, chunks, tag):
        """Accumulate psum += src^T-tile_k.T @ rhs_k, interleaving the PE
        transposes with the accumulation matmuls (same-engine slot safety)."""
        for k in range(nk):
            tp = pe_T(src_tile[:, k * 128:(k + 1) * 128], 128)
            tt = sb.tile([128, 128], f32, tag=tag)
            nc.vector.tensor_copy(tt[:], tp[:])
            for c0, c1 in chunks:
                nc.tensor.matmul(psum[:, c0:c1], tt[:], rhs_fn(k)[:, c0:c1],
                                 start=(k == 0), stop=(k == nk - 1))

    # xt = target @ fcxtW + b  (ones col at 1000)
    tg = sb.tile([128, 1024], f32, tag="tg")
    nc.sync.dma_start(tg[:, 0:1000], tgtD[:])
    nc.vector.memset(tg[:, 1000:1001], 1.0)
    nc.vector.memset(tg[:, 1001:1024], 0.0)
    fcxtWk = sb.tile([128, 8, 128], f32, tag="tw8")
    nc.sync.dma_start(fcxtWk[:], fcxtWkD.ap().rearrange("k r n -> r k n"))
    xt_ps = ps.tile([128, 128], f32, tag="ps_small")
    mm_transposed(tg, 8, lambda k: fcxtWk[:, k, :], xt_ps, ((0, 128),), "ttl")
    xt_sb = sb.tile([128, 128], f32, tag="xt")
    nc.vector.tensor_copy(xt_sb[:], xt_ps[:])

    # per-branch g vectors
    gvecs = []
    for bi, p in enumerate(("p1", "p2")):
        fg1D = d(p + "_fcg1Wk", [19, 128, 1024], f32, kind="ExternalInput")
        fg2D = d(p + "_fcg2Wk", [8, 128, 64], f32, kind="ExternalInput")
        fg1 = sb.tile([128, 19, 1024], f32, tag="fg1", bufs=1)
        nc.sync.dma_start(fg1[:], fg1D.ap().rearrange("k r n -> r k n"))
        g_ps = ps.tile([128, 1024], f32, tag="ps_big")
        kts = list(range(bi * 18, bi * 18 + 18)) + [36]
        for k, kt in enumerate(kts):
            pl0 = sb.tile([128, 128], f32, tag="pl0")
            ld = nc.sync.dma_start(pl0[:], pool_glob[:, kt * 128:(kt + 1) * 128])
            add_dep_helper(ld.ins, ag_pool.ins, reason="pool load after AG")
            tp = pe_T(pl0[:], 128)
            pl = sb.tile([128, 128], f32, tag="plt")
            nc.vector.tensor_copy(pl[:], tp[:])
            for c0, c1 in ((0, 512), (512, 1024)):
                nc.tensor.matmul(g_ps[:, c0:c1], pl[:], fg1[:, k, c0:c1],
                                 start=(k == 0), stop=(k == 18))
        glr = sb.tile([128, 1024], f32, tag="glr")
        nc.scalar.activation(glr[:, 0:1000], g_ps[:, 0:1000],
                             mybir.ActivationFunctionType.Lrelu, alpha=0.01)
        nc.vector.memset(glr[:, 1000:1001], 1.0)
        nc.vector.memset(glr[:, 1001:1024], 0.0)
        fg2 = sb.tile([128, 8, 64], f32, tag="tw8b")
        nc.sync.dma_start(fg2[:], fg2D.ap().rearrange("k r n -> r k n"))
        g2_ps = ps.tile([128, 64], f32, tag="ps_small")
        mm_transposed(glr, 8, lambda k: fg2[:, k, :], g2_ps, ((0, 64),), "gtl")
        gv = sb.tile([128, 64], f32, tag=f"gv{bi}")
        nc.vector.tensor_copy(gv[:], g2_ps[:])
        gvecs.append(gv)

    # xcT k-tiles
    xcT0 = sb.tile([128, 128], f32, tag="xcT0")
    t0 = pe_T(gvecs[0][:], 64)
    nc.vector.tensor_copy(xcT0[0:64, :], t0[:])
    t1 = pe_T(gvecs[1][:], 64)
    nc.vector.tensor_copy(xcT0[64:128, :], t1[:])
    xcT1 = sb.tile([128, 128], f32, tag="xcT1")
    t2 = pe_T(xt_sb[:], 128)
    nc.vector.tensor_copy(xcT1[:], t2[:])
    ones = sb.tile([128, 128], f32, tag="ones")
    nc.vector.memset(ones[:], 0.0)
    nc.vector.memset(ones[0:1, :], 1.0)

    fc1Wk = sb.tile([128, 3, 128], f32, tag="fc1w")
    nc.sync.dma_start(fc1Wk[:], fc1WkD.ap().rearrange("k r n -> r k n"))
    xc1_ps = ps.tile([128, 128], f32, tag="ps_small")
    for k, lt in enumerate((xcT0, xcT1, ones)):
        nc.tensor.matmul(xc1_ps[:], lt[:], fc1Wk[:, k, :], start=(k == 0), stop=(k == 2))
    xc1 = sb.tile([128, 128], f32, tag="xc1")
    nc.scalar.activation(xc1[:], xc1_ps[:],
                         mybir.ActivationFunctionType.Lrelu, alpha=0.01)
    xc1T = sb.tile([128, 128], f32, tag="xc1T")
    t3 = pe_T(xc1[:], 128)
    nc.vector.tensor_copy(xc1T[:], t3[:])

    fc2Wk = sb.tile([128, 2, 32], f32, tag="fc2w")
    nc.sync.dma_start(fc2Wk[:], fc2WkD.ap().rearrange("k r n -> r k n"))
    xc2_ps = ps.tile([128, 32], f32, tag="ps_small")
    for k, lt in enumerate((xc1T, ones)):
        nc.tensor.matmul(xc2_ps[:], lt[:], fc2Wk[:, k, :], start=(k == 0), stop=(k == 1))
    xc2 = sb.tile([128, 32], f32, tag="xc2")
    nc.scalar.activation(xc2[:], xc2_ps[:],
                         mybir.ActivationFunctionType.Lrelu, alpha=0.01)
    xc2T = sb.tile([128, 128], f32, tag="xc2T")
    nc.vector.memset(xc2T[:], 0.0)
    t4 = pe_T(xc2[:], 32)
    nc.vector.tensor_copy(xc2T[0:32, :], t4[:])
    nc.vector.memset(xc2T[32:33, :], 1.0)

    outWk = sb.tile([128, 1], f32, tag="outw")
    nc.sync.dma_start(outWk[:], outWkD[:])
    out_ps = ps.tile([128, 1], f32, tag="ps_small")
    nc.tensor.matmul(out_ps[:], xc2T[:], outWk[:], start=True, stop=True)
    outsb = sb.tile([128, 1], f32, tag="outsb")
    nc.vector.tensor_copy(outsb[:], out_ps[:])
    nc.sync.dma_start(outD[:], outsb[:])


def _build_program(st1, st2):
    nc = bacc.Bacc("TRN2", target_bir_lowering=False, debug=False, num_devices=NC)
    d = nc.dram_tensor
    identD = d("identity", [128, 128], f32, kind="ExternalInput")
    pool_loc = d("pool_loc", [GPC, 4736], f32)
    pool_glob = d("pool_glob", [G, 4736], f32, addr_space="Shared")

    with tile.TileContext(nc) as tc:
        with (
            tc.tile_pool(name="base", bufs=1) as base,
            tc.tile_pool(name="ps", bufs=2, space="PSUM") as ps,
        ):
            lib = nc.gpsimd.load_library(library_config.mlp)
            ident = base.tile([128, 128], f32, tag="ident")
            nc.sync.dma_start(ident[:], identD[:])
            identb = base.tile([128, 128], bf16, tag="identb")
            nc.vector.tensor_copy(identb[:], ident[:])
            pools = {"ps": ps, "ident": ident, "identb": identb, "lib": lib}
            ctx = None

            # zero the pooled staging buffer (cols 4608 bias=1, rest padded 0)
            stg0 = base.tile([GPC, 4736], f32, tag="stg0")
            nc.vector.memset(stg0[:], 0.0)
            nc.vector.memset(stg0[:, 4608:4609], 1.0)
            plw = nc.sync.dma_start(pool_loc[:], stg0[:])

            brs = []
            for pfx, st in (("b1_", st1), ("b2_", st2)):
                with tc.tile_pool(name="gat" + pfx, bufs=2) as sb:
                    pools["sb"] = sb
                    brs.append(_build_branch(nc, tc, ctx, pools, pfx, st, {}))
            ws = []
            for br, col0 in ((brs[0], 0), (brs[1], 2304)):
                with tc.tile_pool(name="gcn" + br["pfx"], bufs=2) as sb:
                    pools["sb"] = sb
                    ws.extend(_build_gcn(nc, tc, ctx, pools, br, pool_loc, col0, plw))
            ag_pool = nc.gpsimd.collective_compute(
                "AllGather", mybir.AluOpType.bypass,
                replica_groups=[list(range(NC))],
                ins=[pool_loc[:]], outs=[pool_glob[:]])
            add_dep_helper(ag_pool.ins, plw.ins, reason="AG pool after init")
            for w in ws:
                add_dep_helper(ag_pool.ins, w.ins, reason="AG pool after stage writes")
            with tc.tile_pool(name="tail", bufs=2) as sb:
                pools["sb"] = sb
                _build_tail(nc, tc, ctx, pools, pool_glob, ag_pool)

    nc.compile()
    return nc


# ---------------------------------------------------------------- entry point
def kernel(**inputs) -> np.ndarray:
    st1, pc1, sh1 = _prep_branch(inputs["x1"], inputs["edge_index1"], inputs["batch1"],
                                 inputs["p1_gatW"], inputs["p1_att_src"],
                                 inputs["p1_att_dst"])
    st2, pc2, sh2 = _prep_branch(inputs["x2"], inputs["edge_index2"], inputs["batch2"],
                                 inputs["p2_gatW"], inputs["p2_att_src"],
                                 inputs["p2_att_dst"])
    gatWk1, gcnWk1 = _pack_branch_weights(inputs["p1_gatW"], inputs["p1_gatb"],
                                          inputs["p1_gcnW"], inputs["p1_gcnb"])
    gatWk2, gcnWk2 = _pack_branch_weights(inputs["p2_gatW"], inputs["p2_gatb"],
                                          inputs["p2_gcnW"], inputs["p2_gcnb"])
    tail = _pack_tail(inputs)

    key = (st1["MAXG"], st1["NSUB"], st2["MAXG"], st2["NSUB"])
    if key not in _PROG_CACHE:
        _PROG_CACHE[key] = _build_program(st1, st2)
    nc = _PROG_CACHE[key]

    in_maps = []
    for c in range(NC):
        m = {"identity": tail["identity"], "target": tail["target"],
             "fcxtWk": tail["fcxtWk"], "fc1Wk": tail["fc1Wk"],
             "fc2Wk": tail["fc2Wk"], "outWk": tail["outWk"],
             "p1_fcg1Wk": tail["p1_fcg1Wk"], "p1_fcg2Wk": tail["p1_fcg2Wk"],
             "p2_fcg1Wk": tail["p2_fcg1Wk"], "p2_fcg2Wk": tail["p2_fcg2Wk"]}
        for pfx, pc, sh, gatWk, gcnWk in (("b1_", pc1, sh1, gatWk1, gcnWk1),
                                          ("b2_", pc2, sh2, gatWk2, gcnWk2)):
            p = pc[c]
            m[pfx + "xpad"] = sh["xpad"]
            m[pfx + "Wsd"] = sh["Wsd"]
            m[pfx + "gatWk"] = gatWk
            m[pfx + "gcnWk"] = gcnWk
            for k in ("isrc", "mask", "maskT", "xidx", "dinv", "valid", "inval",
                      "poison", "rcnt"):
                m[pfx + k] = p[k]
        in_maps.append(m)

    res = run_bass_kernel_spmd(nc, in_maps, list(range(NC)))
    return np.asarray(res.results[0]["out"], dtype=np.float32)



# revision 30
# speedup vs baseline: 2.0114x; 2.0114x over previous
"""Trainium2 Bass kernel for nn_GAT_GCN (GAT conv + GCN conv + pooling + MLP tail).

Strategy (8 NeuronCores, SPMD, full inputs in / full output out):
  - Nodes sharded by graph: core c owns graphs [16c, 16c+16), laid out in
    graph-aligned slots (MAXG per graph) so pooling reduces over fixed ranges.
  - Edges sharded by destination; per 128-slot destination block, edges are
    bucketed into a variable number of 128-edge subtiles (per-block count =
    max over cores, so the program is uniform). Self-loops are handled densely
    (identity-mask / local-row adds), not in the edge buckets.
  - GAT: per-edge payload [x[src] | a_src[src]] (256 B rows) gathered from an
    AllGathered table; exp(e) folded into the mask matmul; softmax/dinv applied
    on the Scalar engine when producing z. The head-blocked W multiply is fused
    into the same block loop via PE transposes (no DRAM round trip).
  - GCN: z rows (pre-scaled by dinv[src]) gathered as matmul rhs; self rows
    loaded directly from local z. The dense [1140x1152] W multiply runs fused
    per 4-block group via PE transposes, producing zfin^T chunks that feed
    incremental max/mean pooling. No pooled AllGather: the MLP tail runs
    per-core on its 16 graphs; only the final [16,1] output is AllGathered.
  - Branch-2 GAT (DVE-heavy) is interleaved block-by-block with branch-1 GCN
    (DMA/PE-heavy) to overlap engine usage; AllGathers overlap compute.

Host-side preprocessing is restricted to index manipulation (sorting/bucketing
edges, one-hot masks, padding) and parameter repacking (padding / bf16 casts /
tiny reshapes of weights) -- all data-dependent float compute runs on device.
"""

import math
import os
from contextlib import ExitStack

import numpy as np
import ml_dtypes

import concourse.bacc as bacc
import concourse.bass as bass
import concourse.tile as tile
from concourse import mybir, library_config
from concourse.bass_utils import run_bass_kernel_spmd
from concourse.tile_rust import add_dep_helper

# ---------------------------------------------------------------- constants
N = 20000
E = 160000
G = 128
F = 114
H = 10
HF = 1140          # F * H
NC = 8
GPC = G // NC      # graphs per core
P = 128

bf16 = mybir.dt.bfloat16
f32 = mybir.dt.float32
f32r = mybir.dt.float32r
i16 = mybir.dt.int16

BF = ml_dtypes.bfloat16

_PROG_CACHE: dict = {}
LAST_RESULTS = None  # debug: BassKernelResults of the most recent run


# ---------------------------------------------------------------- host utils
def _wrap_idx(idx: np.ndarray) -> np.ndarray:
    """int16 index list (len % 16 == 0) -> [128, len/16] wrapped layout."""
    n = idx.shape[0]
    assert n % 16 == 0
    return np.tile(idx.reshape(-1, 16).T, (8, 1)).astype(np.int16)


def _prep_branch(x, ei, batch, gatW, att_src, att_dst):
    """Host preprocessing for one branch. Returns (static, percore, shared)."""
    x = np.asarray(x, dtype=np.float32)
    ei = np.asarray(ei).astype(np.int64)
    batch = np.asarray(batch).astype(np.int64)
    gatW = np.asarray(gatW, dtype=np.float32)
    att_src = np.asarray(att_src, dtype=np.float32)
    att_dst = np.asarray(att_dst, dtype=np.float32)

    cnt = np.bincount(batch, minlength=G)
    MAXG = int(cnt.max())
    SS = ((16 * MAXG + 127) // 128) * 128       # slots per core shard
    NBLK = SS // 128
    assert NC * SS < 32768, "row ids must fit int16"

    gstart = np.zeros(G + 1, np.int64)
    np.cumsum(cnt, out=gstart[1:])
    nodes = np.arange(N)
    rank = nodes - gstart[batch]
    slot_of_node = (batch % GPC) * MAXG + rank          # [N] in [0, 16*MAXG)
    core_of_node = batch // GPC                          # [N]
    row_of_node = core_of_node * SS + slot_of_node       # [N] global table row

    # real edges only; self loops handled densely on device
    src = ei[0]
    dst = ei[1]

    core_e = core_of_node[dst]
    slot_e = slot_of_node[dst]
    blk_e = slot_e // 128
    dloc_e = slot_e % 128

    # bucket edges per (core, block)
    order = np.lexsort((blk_e, core_e))
    src_s, core_s, blk_s, dloc_s = src[order], core_e[order], blk_e[order], dloc_e[order]
    key = core_s * NBLK + blk_s
    counts = np.bincount(key, minlength=NC * NBLK)
    starts = np.zeros(NC * NBLK + 1, np.int64)
    np.cumsum(counts, out=starts[1:])
    cm = counts.reshape(NC, NBLK)
    nsub = np.maximum(1, -(-cm.max(axis=0) // 128))      # per-block subtiles
    nsub = [int(v) for v in nsub]
    nidx = [int(max(16, -(-int(v) // 16) * 16)) for v in cm.max(axis=0)]
    sub_off = np.zeros(NBLK + 1, np.int64)
    np.cumsum(nsub, out=sub_off[1:])
    TOT_SUB = int(sub_off[-1])

    # degree with self loop (reference adds self loops before degree calc)
    deg = np.bincount(dst, minlength=N).astype(np.float64) + 1.0
    dinv_node = 1.0 / np.sqrt(deg)

    percore = []
    for c in range(NC):
        isrc = np.zeros((128, TOT_SUB * 8), np.int16)
        mask = np.zeros((128, TOT_SUB, 128), np.float32)      # [e, sub, dloc]
        for b in range(NBLK):
            k = c * NBLK + b
            cnt_b = counts[k]
            cap_b = nsub[b] * 128
            sl = slice(starts[k], starts[k + 1])
            idx = np.zeros(cap_b, np.int64)
            idx[:cnt_b] = row_of_node[src_s[sl]]
            isrc[:, sub_off[b] * 8:sub_off[b + 1] * 8] = _wrap_idx(idx.astype(np.int16))
            pos = np.arange(cnt_b)
            mask[pos % 128, sub_off[b] + pos // 128, dloc_s[sl]] = 1.0
        maskT = np.ascontiguousarray(mask.transpose(2, 1, 0))  # [d, sub, e]

        # per-slot metadata for this core
        slots = np.arange(SS)
        g_local = np.minimum(slots // MAXG, GPC - 1)
        r_local = slots - g_local * MAXG
        g_global = c * GPC + g_local
        validity = (slots < 16 * MAXG) & (r_local < cnt[g_global])
        node_of_slot = np.zeros(SS, np.int64)
        real = validity.nonzero()[0]
        node_of_slot[real] = gstart[g_global[real]] + r_local[real]
        xslot = np.zeros((SS, 128), np.float32)
        xslot[real, :F] = x[node_of_slot[real]]

        dinv_slot = np.zeros(SS, np.float32)                 # 0 on invalid slots
        dinv_slot[real] = dinv_node[node_of_slot[real]]
        valid = validity.astype(np.float32)
        dvi = np.stack([dinv_slot, valid, 1.0 - valid], axis=1)   # [SS, 3]
        poison = np.where(validity, 0.0, -1e28).astype(np.float32)
        rcnt = (1.0 / np.maximum(cnt[c * GPC:(c + 1) * GPC], 1.0)).astype(np.float32)

        percore.append({
            "isrc": isrc,                                       # [128, TOT_SUB*8]
            "mask": mask.astype(BF),                            # [128, TOT_SUB, 128]
            "maskT": maskT.astype(BF),
            "xslot": xslot.astype(BF),                          # [SS, 128]
            "dvi": dvi.reshape(NBLK, 128, 3),                   # f32
            "poison": np.broadcast_to(poison.astype(BF), (128, SS)).copy(),
            "rcnt": np.broadcast_to(rcnt, (128, GPC)).copy(),
        })

    # shared (weights)
    gatW3 = gatW.reshape(F, H, F)
    Ws = np.einsum("khf,hf->kh", gatW3, att_src)
    Wd = np.einsum("khf,hf->kh", gatW3, att_dst)
    Wsd = np.zeros((128, 2 * H), np.float32)
    Wsd[:F, :H] = Ws
    Wsd[:F, H:] = Wd

    shared = {
        "Wsd": Wsd.astype(BF),                # [128,20]
    }
    static = dict(MAXG=MAXG, SS=SS, NBLK=NBLK, nsub=nsub, nidx=nidx,
                  sub_off=[int(v) for v in sub_off], TOT_SUB=TOT_SUB)
    return static, percore, shared


def _pack_branch_weights(gatW, gatb, gcnW, gcnb):
    gatW = np.asarray(gatW, np.float32)
    gatb = np.asarray(gatb, np.float32)
    gcnW = np.asarray(gcnW, np.float32)
    gcnb = np.asarray(gcnb, np.float32)
    gatWk = np.zeros((H, 128, F), np.float32)
    gatW3 = gatW.reshape(F, H, F)
    for h in range(H):
        gatWk[h, :F, :] = gatW3[:, h, :]
        gatWk[h, F, :] = gatb[h * F:(h + 1) * F]
    gcn_pad = np.zeros((1152, 1152), np.float32)
    gcn_pad[:HF, :HF] = gcnW
    gcn_pad[HF, :HF] = gcnb
    gcnWk = gcn_pad.reshape(9, 128, 1152)
    return gatWk.astype(BF), gcnWk.astype(BF)


def _r32(a):
    """Round to FP32r (truncate mantissa to 13 bits) for f32r matmuls."""
    u = np.ascontiguousarray(a, dtype=np.float32).view(np.uint32)
    return (u & np.uint32(0xFFFFFC00)).view(np.float32)


def _pack_tail(inp):
    f = lambda k: np.asarray(inp[k], np.float32)
    tail = {}
    for p in ("p1", "p2"):
        W1 = np.zeros((2432, 1024), np.float32)
        fg1 = f(p + "_fcg1W")           # [2280, 1000]
        W1[0:HF, 0:1000] = fg1[0:HF]
        W1[1152:1152 + HF, 0:1000] = fg1[HF:2 * HF]
        k1 = np.concatenate([W1[:2304].reshape(18, 128, 1024),
                             np.zeros((1, 128, 1024), np.float32)], axis=0)
        k1[18, 0, 0:1000] = f(p + "_fcg1b")
        tail[p + "_fcg1Wk"] = _r32(np.stack([k1[:, :, 0:512], k1[:, :, 512:1024]]))
        W2 = np.zeros((1024, 64), np.float32)
        W2[0:1000] = f(p + "_fcg2W")
        W2[1000] = f(p + "_fcg2b")
        tail[p + "_fcg2Wk"] = _r32(W2.reshape(8, 128, 64))
    Wx = np.zeros((1024, 128), np.float32)
    Wx[0:1000] = f("fcxtW")
    Wx[1000] = f("fcxtb")
    tail["fcxtWk"] = Wx.reshape(8, 128, 128).astype(np.float32)
    W1 = np.zeros((3, 128, 128), np.float32)
    W1[0] = f("fc1W")[0:128]
    W1[1] = f("fc1W")[128:256]
    W1[2, 0] = f("fc1b")
    tail["fc1Wk"] = W1.astype(np.float32)
    W2 = np.zeros((2, 128, 32), np.float32)
    W2[0] = f("fc2W")
    W2[1, 0] = f("fc2b")
    tail["fc2Wk"] = W2.astype(np.float32)
    Wo = np.zeros((128, 1), np.float32)
    Wo[0:32, 0] = f("outW")[:, 0]
    Wo[32, 0] = float(np.asarray(inp["outb"]).reshape(-1)[0])
    tail["outWk"] = Wo.astype(np.float32)
    tail["identity"] = np.eye(128, dtype=np.float32)
    tail["target"] = f("target")
    return tail


# ---------------------------------------------------------------- device build
GATHER_MAX = 1024  # dma_gather breaks above 1024 indices per call (HW-probed)


def _gather_chunked(nc, pools, out_tile, table_ap, ii, nrows, elem, deps=()):
    """Emit dma_gather in <=1024-index chunks; nrows is a multiple of 16."""
    insts = []
    for i0 in range(0, nrows, GATHER_MAX):
        i1 = min(nrows, i0 + GATHER_MAX)
        n = i1 - i0
        g = nc.gpsimd.dma_gather(
            out_tile[:, i0 // 128:(i1 + 127) // 128, :], table_ap,
            ii[:, i0 // 16:i1 // 16], n, n, elem)
        add_dep_helper(g.ins, pools["lib"].ins, reason="gather after lib")
        for dd in deps:
            add_dep_helper(g.ins, dd.ins, reason="gather dep")
        insts.append(g)
    return insts


class _Branch:
    """Holds per-branch DRAM handles + SBUF tiles that span sections."""


def _decl_branch(nc, pools, pfx, st):
    br = _Branch()
    br.pfx = pfx
    br.st = st
    SS, NBLK, TOT_SUB = st["SS"], st["NBLK"], st["TOT_SUB"]
    d = nc.dram_tensor
    br.xslot = d(pfx + "xslot", [SS, 128], bf16, kind="ExternalInput")
    br.isrc = d(pfx + "isrc", [128, TOT_SUB * 8], i16, kind="ExternalInput")
    br.maskD = d(pfx + "mask", [128, TOT_SUB, 128], bf16, kind="ExternalInput")
    br.maskTD = d(pfx + "maskT", [128, TOT_SUB, 128], bf16, kind="ExternalInput")
    br.dviD = d(pfx + "dvi", [NBLK, 128, 3], f32, kind="ExternalInput")
    br.WsdD = d(pfx + "Wsd", [128, 2 * H], bf16, kind="ExternalInput")
    br.gatWkD = d(pfx + "gatWk", [H, 128, F], bf16, kind="ExternalInput")
    br.gcnWkD = d(pfx + "gcnWk", [9, 128, 1152], bf16, kind="ExternalInput")
    br.poisonD = d(pfx + "poison", [128, SS], bf16, kind="ExternalInput")
    br.rcntD = d(pfx + "rcnt", [128, GPC], f32, kind="ExternalInput")
    br.T_gath = d(pfx + "T_gath", [SS, 128], bf16)
    br.T_glob = d(pfx + "T_glob", [NC * SS, 128], bf16, addr_space="Shared")
    br.z_loc = d(pfx + "z_loc", [SS, 1152], bf16)
    br.z_glob = d(pfx + "z_glob", [NC * SS, 1152], bf16, addr_space="Shared")
    # cross-section SBUF: T band (x | a_s | a_d | pad), per-block metadata
    base = pools["base"]
    br.Tband = base.tile([128, NBLK, 144], bf16, tag=pfx + "Tband")
    br.t_writes = []
    br.z_writes = []
    return br


def _build_tbuild(nc, pools, br, sb):
    """Phase 1: build T rows (x | a_s | a_d), write gatherable part to DRAM."""
    st = br.st
    NBLK = st["NBLK"]
    ps = pools["ps"]
    Wsd = sb.tile([128, 2 * H], bf16, tag="wsd")
    nc.sync.dma_start(Wsd[:], br.WsdD[:])
    identb = pools["identb"]
    for b in range(NBLK):
        xg = sb.tile([128, 128], bf16, tag="xg")
        nc.sync.dma_start(xg[:], br.xslot[b * 128:(b + 1) * 128, :])
        tp = ps.tile([128, 128], bf16, tag="tp")
        nc.tensor.transpose(tp[:], xg[:], identb[:])
        xgT = sb.tile([128, 128], bf16, tag="xgT")
        nc.scalar.copy(xgT[:], tp[:])
        aps_t = ps.tile([128, 128], f32, tag="tp")
        aps = aps_t[:, 0:2 * H]
        nc.tensor.matmul(aps[:], xgT[:], Wsd[:], start=True, stop=True)
        # T band: [x(114) | a_s(10) | a_d(10) | pad]
        nc.vector.tensor_copy(br.Tband[:, b, 0:F], xg[:, 0:F])
        nc.vector.tensor_copy(br.Tband[:, b, F:F + 2 * H], aps[:])
        nc.vector.memset(br.Tband[:, b, F + 2 * H:144], 0.0)
        # gatherable table row: [x(114) | a_s(10) | pad(4)] = 128 cols
        Tg = sb.tile([128, 128], bf16, tag="Tg")
        nc.vector.tensor_copy(Tg[:, 0:F + H], br.Tband[:, b, 0:F + H])
        nc.vector.memset(Tg[:, F + H:128], 0.0)
        w = nc.sync.dma_start(br.T_gath[b * 128:(b + 1) * 128, :], Tg[:])
        br.t_writes.append(w)


def _ag_t(nc, br):
    ag = nc.gpsimd.collective_compute(
        "AllGather", mybir.AluOpType.bypass,
        replica_groups=[list(range(NC))],
        ins=[br.T_gath[:]], outs=[br.T_glob[:]])
    for w in br.t_writes:
        add_dep_helper(ag.ins, w.ins, reason="AG_T after T writes")
    br.ag_t = ag


def _ag_z(nc, br):
    ag = nc.gpsimd.collective_compute(
        "AllGather", mybir.AluOpType.bypass,
        replica_groups=[list(range(NC))],
        ins=[br.z_loc[:]], outs=[br.z_glob[:]])
    for w in br.z_writes:
        add_dep_helper(ag.ins, w.ins, reason="AG_z after z writes")
    br.ag_z = ag


def _gat_weights(nc, br, sb):
    br.gatWk = sb.tile([128, H, F], bf16, tag="gatwk", bufs=1)
    nc.sync.dma_start(br.gatWk[:], br.gatWkD.ap().rearrange("h k n -> k h n"))


def _gat_load(nc, pools, br, sb, b):
    """Gather prologue for one GAT block (gpsimd side only)."""
    st = br.st
    ns = st["nsub"][b]
    so = st["sub_off"][b]
    cap = ns * 128
    ii = sb.tile([128, ns * 8], i16, tag="ii", bufs=8)
    nc.sync.dma_start(ii[:], br.isrc[:, so * 8:(so + ns) * 8])
    S = sb.tile([128, ns, 128], bf16, tag="S", bufs=6)
    _gather_chunked(nc, pools, S, br.T_glob[:], ii, cap, 128, deps=(br.ag_t,))
    return S


def _build_gat_block(nc, pools, br, sb, b, S):
    """Fused GAT block: attention + mask matmul + W pass -> z_loc."""
    st = br.st
    ns = st["nsub"][b]
    so = st["sub_off"][b]
    ps = pools["ps"]
    identb = pools["identb"]

    Mt = sb.tile([128, ns, 128], bf16, tag="Mt")
    nc.sync.dma_start(Mt[:], br.maskD[:, so:so + ns, :])
    MtT = sb.tile([128, ns, 128], bf16, tag="MtT")
    nc.sync.dma_start(MtT[:], br.maskTD[:, so:so + ns, :])
    dvi = sb.tile([128, 3], f32, tag="dvi")
    nc.sync.dma_start(dvi[:], br.dviD[b])

    # per-edge attention logits: a_s from gather, a_d via maskT matmul
    ad_t = ps.tile([128, 128], f32, tag="tp")
    adv = ad_t[:, 0:ns * H].rearrange("p (s h) -> p s h", h=H)
    for s in range(ns):
        nc.tensor.matmul(adv[:, s, :], MtT[:, s, :],
                         br.Tband[:, b, F + H:F + 2 * H], start=True, stop=True)
    lg = sb.tile([128, ns, H], f32, tag="lg")
    nc.vector.tensor_tensor(out=lg[:], in0=S[:, :, F:F + H], in1=adv[:],
                            op=mybir.AluOpType.add)
    l3 = sb.tile([128, ns, H], f32, tag="l3")
    nc.vector.scalar_tensor_tensor(out=l3[:], in0=lg[:], scalar=0.2, in1=lg[:],
                                   op0=mybir.AluOpType.mult,
                                   op1=mybir.AluOpType.max)
    exb = sb.tile([128, ns, H], bf16, tag="exb")
    nc.scalar.activation(exb[:], l3[:], mybir.ActivationFunctionType.Exp)

    # self-loop attention (own slots)
    lgs = sb.tile([128, H], f32, tag="lgs")
    nc.vector.tensor_tensor(out=lgs[:], in0=br.Tband[:, b, F:F + H],
                            in1=br.Tband[:, b, F + H:F + 2 * H],
                            op=mybir.AluOpType.add)
    l3s = sb.tile([128, H], f32, tag="l3s")
    nc.vector.scalar_tensor_tensor(out=l3s[:], in0=lgs[:], scalar=0.2, in1=lgs[:],
                                   op0=mybir.AluOpType.mult,
                                   op1=mybir.AluOpType.max)
    exs = sb.tile([128, H], bf16, tag="exs")
    nc.scalar.activation(exs[:], l3s[:], mybir.ActivationFunctionType.Exp)

    # exp-weighted payloads
    R = sb.tile([128, ns, HF], bf16, tag="R")
    nc.vector.tensor_tensor(
        out=R[:].rearrange("p s (h f) -> p s h f", h=H),
        in0=S[:, :, 0:F].unsqueeze(2).broadcast_to([128, ns, H, F]),
        in1=exb[:].unsqueeze(3).broadcast_to([128, ns, H, F]),
        op=mybir.AluOpType.mult)
    Rs = sb.tile([128, HF], bf16, tag="Rs")
    nc.vector.tensor_tensor(
        out=Rs[:].rearrange("p (h f) -> p h f", h=H),
        in0=br.Tband[:, b, 0:F].unsqueeze(1).broadcast_to([128, H, F]),
        in1=exs[:].unsqueeze(2).broadcast_to([128, H, F]),
        op=mybir.AluOpType.mult)

    # aggregation: y[d, :] = sum_e mask * R   (+ identity-mask self term);
    # softmax denominator via the same masks on the exp weights
    big = ps.tile([128, 1280], f32, tag="big")
    y_ps = big[:, 0:HF]
    for c0, c1 in ((0, 512), (512, 1024), (1024, HF)):
        nc.tensor.matmul(y_ps[:, c0:c1], identb[:], Rs[:, c0:c1],
                         start=True, stop=(ns == 0))
    for s in range(ns):
        for c0, c1 in ((0, 512), (512, 1024), (1024, HF)):
            nc.tensor.matmul(y_ps[:, c0:c1], Mt[:, s, :], R[:, s, c0:c1],
                             start=False, stop=(s == ns - 1))
    dps_t = ps.tile([128, 128], f32, tag="tp")
    den_ps = dps_t[:, 0:H]
    for s in range(ns):
        nc.tensor.matmul(den_ps[:], Mt[:, s, :], exb[:, s, :],
                         start=(s == 0), stop=(s == ns - 1))

    # den = mask-sum + self + invalid guard; dinv folding
    den_b = sb.tile([128, H], f32, tag="den_b")
    nc.vector.tensor_tensor(out=den_b[:], in0=den_ps[:], in1=exs[:],
                            op=mybir.AluOpType.add)
    nc.vector.tensor_scalar(out=den_b[:], in0=den_b[:], scalar1=dvi[:, 2:3],
                            scalar2=None, op0=mybir.AluOpType.add)
    rden = sb.tile([128, H], f32, tag="rden")
    nc.vector.reciprocal(rden[:], den_b[:])
    rden2 = sb.tile([128, H], f32, tag="rden2")
    nc.vector.tensor_scalar(out=rden2[:], in0=rden[:], scalar1=dvi[:, 0:1],
                            scalar2=None, op0=mybir.AluOpType.mult)

    # assemble y tile (numerator | den at col F per head) on Scalar engine
    yt = sb.tile([128, 1280], bf16, tag="yt")
    ytv = yt[:].rearrange("p (h c) -> p h c", h=H)
    nc.scalar.copy(ytv[:, :, 0:F],
                   y_ps[:].rearrange("p (h f) -> p h f", h=H))
    nc.scalar.copy(ytv[:, :, F:F + 1], den_b[:].unsqueeze(2))
    nc.vector.memset(ytv[:, :, F + 1:128], 0.0)

    # W pass: transpose head chunks on PE into one psum tile, one ACT copy
    tpy = ps.tile([128, 1280], bf16, tag="big")
    for h in range(H):
        nc.tensor.transpose(tpy[:, h * 128:(h + 1) * 128],
                            yt[:, h * 128:(h + 1) * 128], identb[:])
    yT = sb.tile([128, H, 128], bf16, tag="yT", bufs=1)
    nc.scalar.copy(yT[:].rearrange("p h c -> p (h c)"), tpy[:])
    zp = ps.tile([128, 1280], f32, tag="big")
    for h in range(H):
        first = (h % 4 == 0)
        last = (h % 4 == 3) or (h == H - 1)
        nc.tensor.matmul(zp[:, h * 128:h * 128 + F], yT[:, h, :],
                         br.gatWk[:, h, :], start=first, stop=last)

    # z = lrelu(zp * rden2) per head on the Scalar engine (DVE is the
    # bottleneck; lrelu(s*x) == s*lrelu(x) for s >= 0)
    zt = sb.tile([128, 1152], bf16, tag="zt")
    for h in range(H):
        nc.scalar.activation(zt[:, h * F:(h + 1) * F], zp[:, h * 128:h * 128 + F],
                             mybir.ActivationFunctionType.Lrelu, alpha=0.01,
                             scale=rden2[:, h:h + 1])
    nc.vector.memset(zt[:, HF:1152], 0.0)
    w = nc.sync.dma_start(br.z_loc[b * 128:(b + 1) * 128, :], zt[:])
    br.z_writes.append(w)


def _gcn_weights(nc, br, sb):
    st = br.st
    br.gcnWk = sb.tile([128, 9, 1152], bf16, tag="gcnwk", bufs=1)
    nc.sync.dma_start(br.gcnWk[:], br.gcnWkD.ap().rearrange("kb kr n -> kr kb n"))
    br.rcnt = sb.tile([128, GPC], f32, tag="rcnt", bufs=1)
    nc.sync.dma_start(br.rcnt[:], br.rcntD[:])
    br.mxA = sb.tile([128, 9, GPC], f32, tag="mxA", bufs=1)
    nc.vector.memset(br.mxA[:], -1e30)
    br.smA = sb.tile([128, 9, GPC], f32, tag="smA", bufs=1)
    nc.vector.memset(br.smA[:], 0.0)
    br.y2T = None
    br.y2T_fill = 0


def _build_gcn_block(nc, pools, br, sb, b, zbufs=2, ybufs=1):
    """Fused GCN block: gather z + mask matmul + self row; group W + pooling."""
    st = br.st
    MAXG, NBLK = st["MAXG"], st["NBLK"]
    ns = st["nsub"][b]
    so = st["sub_off"][b]
    cap = ns * 128
    ps = pools["ps"]
    identb = pools["identb"]

    ii = sb.tile([128, ns * 8], i16, tag="gii", bufs=zbufs)
    nc.sync.dma_start(ii[:], br.isrc[:, so * 8:(so + ns) * 8])
    Z = sb.tile([128, ns, 1152], bf16, tag="Z", bufs=zbufs)
    _gather_chunked(nc, pools, Z, br.z_glob[:], ii, ns * 128, 1152,
                    deps=(br.ag_z,))
    Mt = sb.tile([128, ns, 128], bf16, tag="gMt")
    nc.sync.dma_start(Mt[:], br.maskD[:, so:so + ns, :])
    dvi = sb.tile([128, 3], f32, tag="gdvi")
    nc.sync.dma_start(dvi[:], br.dviD[b])
    zrow = sb.tile([128, 1152], bf16, tag="zrow")
    r = nc.sync.dma_start(zrow[:], br.z_loc[b * 128:(b + 1) * 128, :])
    add_dep_helper(r.ins, br.z_writes[b].ins, reason="self row after z write")

    big = ps.tile([128, 1280], f32, tag="big")
    y2_ps = big[:, 0:HF]
    for c0, c1 in ((0, 512), (512, 1024), (1024, HF)):
        nc.tensor.matmul(y2_ps[:, c0:c1], identb[:], zrow[:, c0:c1],
                         start=True, stop=(ns == 0))
    for s in range(ns):
        for c0, c1 in ((0, 512), (512, 1024), (1024, HF)):
            nc.tensor.matmul(y2_ps[:, c0:c1], Mt[:, s, :], Z[:, s, c0:c1],
                             start=False, stop=(s == ns - 1))

    # y2 = (mask-agg + self row) * dinv[dst]; bias gate (valid) at col HF
    y2t = sb.tile([128, 1152], bf16, tag="y2t")
    nc.scalar.activation(y2t[:, 0:HF], y2_ps[:],
                         mybir.ActivationFunctionType.Copy, scale=dvi[:, 0:1])
    nc.vector.tensor_copy(y2t[:, HF:HF + 1], dvi[:, 1:2])
    nc.vector.memset(y2t[:, HF + 1:1152], 0.0)

    # accumulate transposed k-chunks for the group W pass
    if br.y2T is None or br.y2T_fill == 4:
        br.y2T = sb.tile([128, 9, 512], bf16, tag="y2T", bufs=ybufs)
        br.y2T_fill = 0
        br.y2T_base = b
    col = br.y2T_fill * 128
    tpz = ps.tile([128, 1280], bf16, tag="big")
    for kb in range(9):
        nc.tensor.transpose(tpz[:, kb * 128:(kb + 1) * 128],
                            y2t[:, kb * 128:(kb + 1) * 128], identb[:])
    nc.scalar.copy(br.y2T[:, :, col:col + 128],
                   tpz[:, 0:1152].rearrange("p (kb c) -> p kb c", kb=9))
    br.y2T_fill += 1

    if br.y2T_fill == 4 or b == NBLK - 1:
        gw = br.y2T_fill * 128
        r0 = br.y2T_base * 128
        # graph pieces covered by slots [r0, r0+gw) (real slots only)
        pieces = []
        lim = 16 * MAXG
        s0 = r0
        while s0 < min(r0 + gw, lim):
            g = s0 // MAXG
            s1 = min((g + 1) * MAXG, r0 + gw, lim)
            pieces.append((g, s0, s1))
            s0 = s1
        pz = sb.tile([128, 512], bf16, tag="pz")
        nc.sync.dma_start(pz[:, 0:gw], br.poisonD[:, r0:r0 + gw])
        zfg = sb.tile([128, 9, 512], bf16, tag="zfg", bufs=1)
        for nb in range(9):
            ct = ps.tile([128, 1280], f32, tag="big")
            for kb in range(9):
                nc.tensor.matmul(ct[:, 0:gw], br.gcnWk[:, kb, nb * 128:(nb + 1) * 128],
                                 br.y2T[:, kb, 0:gw], start=(kb == 0), stop=(kb == 8))
            nc.scalar.activation(zfg[:, nb, 0:gw], ct[:, 0:gw],
                                 mybir.ActivationFunctionType.Lrelu, alpha=0.01)
        # incremental pooling over graph pieces
        for (g, s0, s1) in pieces:
            L = s1 - s0
            o0 = s0 - r0
            red = sb.tile([128, 9, 2], f32, tag="red")
            nc.vector.reduce_sum(red[:, :, 1:2], zfg[:, :, o0:o0 + L],
                                 axis=mybir.AxisListType.X)
            nc.vector.tensor_tensor(
                out=zfg[:, :, o0:o0 + L], in0=zfg[:, :, o0:o0 + L],
                in1=pz[:, o0:o0 + L].unsqueeze(1).broadcast_to([128, 9, L]),
                op=mybir.AluOpType.add)
            nc.vector.reduce_max(red[:, :, 0:1], zfg[:, :, o0:o0 + L],
                                 axis=mybir.AxisListType.X)
            nc.vector.tensor_tensor(out=br.mxA[:, :, g:g + 1], in0=br.mxA[:, :, g:g + 1],
                                    in1=red[:, :, 0:1], op=mybir.AluOpType.max)
            nc.vector.tensor_tensor(out=br.smA[:, :, g:g + 1], in0=br.smA[:, :, g:g + 1],
                                    in1=red[:, :, 1:2], op=mybir.AluOpType.add)


def _t16(nc, ps, ident, src_ap, rows):
    """PE-transpose a [16, rows] tile -> [rows, 16] PSUM tile."""
    tp_t = ps.tile([128, 128], f32, tag="tp")
    tp = tp_t[0:rows, 0:GPC]
    nc.tensor.transpose(tp, src_ap, ident[0:GPC, 0:GPC])
    return tp


def _pool_finalize(nc, pools, br, sb, pool_sb, k0):
    """Mean scaling + stage pooled stats into pool_sb chunks [k0, k0+18)."""
    mnA = sb.tile([128, 9, GPC], f32, tag="mnA")
    nc.vector.tensor_tensor(out=mnA[:], in0=br.smA[:],
                            in1=br.rcnt[:].unsqueeze(1).broadcast_to([128, 9, GPC]),
                            op=mybir.AluOpType.mult)
    for which, statT in ((0, br.mxA), (1, mnA)):
        nc.scalar.copy(pool_sb[:, k0 + which * 9:k0 + which * 9 + 9, :], statT[:])


def _build_xt(nc, pools, sb):
    """xt = target16 @ fcxtW + b  (per-core 16 rows), computed early."""
    d = nc.dram_tensor
    ps = pools["ps"]
    ident = pools["ident"]
    tgtD = d("target16", [GPC, 1000], f32, kind="ExternalInput")
    fcxtWkD = d("fcxtWk", [8, 128, 128], f32, kind="ExternalInput")
    tg = sb.tile([GPC, 1024], f32, tag="tg", bufs=1)
    nc.sync.dma_start(tg[:, 0:1000], tgtD[:])
    nc.vector.memset(tg[:, 1000:1001], 1.0)
    nc.vector.memset(tg[:, 1001:1024], 0.0)
    fcxtWk = sb.tile([128, 8, 128], f32, tag="tw8", bufs=1)
    nc.sync.dma_start(fcxtWk[:], fcxtWkD.ap().rearrange("k r n -> r k n"))
    xt_ps_t = ps.tile([128, 1280], f32, tag="big")
    xt_ps = xt_ps_t[0:GPC, 0:128]
    for k in range(8):
        tp = _t16(nc, ps, ident, tg[:, k * 128:(k + 1) * 128], 128)
        tt = sb.tile([128, GPC], f32, tag="ttl")
        nc.scalar.copy(tt[:], tp)
        nc.tensor.matmul(xt_ps[:], tt[:], fcxtWk[:, k, :], start=(k == 0),
                         stop=(k == 7))
    xt_sb = pools["base"].tile([GPC, 128], f32, tag="xt")
    nc.vector.tensor_copy(xt_sb[:], xt_ps[:])
    return xt_sb


def _build_gvec(nc, pools, sb, pool_sb, bi, fbufs=1):
    """Per-branch g vector [16, 64] from pooled stats (per-core graphs)."""
    d = nc.dram_tensor
    ps = pools["ps"]
    ident = pools["ident"]
    p = ("p1", "p2")[bi]
    fg1D = d(p + "_fcg1Wk", [2, 19, 128, 512], f32r, kind="ExternalInput")
    fg2D = d(p + "_fcg2Wk", [8, 128, 64], f32r, kind="ExternalInput")
    g_ps_t = ps.tile([128, 1280], f32, tag="big")
    g_ps = g_ps_t[0:GPC, 0:1024]
    kts = list(range(bi * 18, bi * 18 + 18)) + [36]
    for half, (c0, c1) in enumerate(((0, 512), (512, 1024))):
        fg1h = sb.tile([128, 19, 512], f32r, tag="fg1", bufs=fbufs)
        nc.sync.dma_start(
            fg1h[:], fg1D.ap()[half].rearrange("k r n -> r k n"))
        for k, kt in enumerate(kts):
            pl = pool_sb[:, kt, :]
            nc.tensor.matmul(g_ps[:, c0:c1], pl, fg1h[:, k, :],
                             start=(k == 0), stop=(k == 18))
    glr = sb.tile([GPC, 1024], f32, tag="glr")
    nc.scalar.activation(glr[:, 0:1000], g_ps[:, 0:1000],
                         mybir.ActivationFunctionType.Lrelu, alpha=0.01)
    nc.vector.memset(glr[:, 1000:1001], 1.0)
    nc.vector.memset(glr[:, 1001:1024], 0.0)
    fg2 = sb.tile([128, 8, 64], f32r, tag="tw8b")
    nc.sync.dma_start(fg2[:], fg2D.ap().rearrange("k r n -> r k n"))
    g2_ps_t = ps.tile([128, 1280], f32, tag="big")
    g2_ps = g2_ps_t[0:GPC, 0:64]
    for k in range(8):
        tp = _t16(nc, ps, ident, glr[:, k * 128:(k + 1) * 128], 128)
        tt = sb.tile([128, GPC], f32r, tag="gtl")
        nc.scalar.copy(tt[:], tp)
        nc.tensor.matmul(g2_ps[:], tt[:], fg2[:, k, :], start=(k == 0), stop=(k == 7))
    gv = pools["base"].tile([GPC, 64], f32, tag=f"gv{bi}")
    nc.vector.tensor_copy(gv[:], g2_ps[:])
    return gv


def _build_tail(nc, pools, sb, xt_sb, gvecs):
    d = nc.dram_tensor
    ps = pools["ps"]
    ident = pools["ident"]
    fc1WkD = d("fc1Wk", [3, 128, 128], f32, kind="ExternalInput")
    fc2WkD = d("fc2Wk", [2, 128, 32], f32, kind="ExternalInput")
    outWkD = d("outWk", [128, 1], f32, kind="ExternalInput")
    outD = d("out", [G, 1], f32, kind="ExternalOutput")
    out_loc = d("out_loc", [GPC, 1], f32)
    out_glob = d("out_glob", [G, 1], f32, addr_space="Shared")

    xcT0 = sb.tile([128, GPC], f32, tag="xcT0")
    t0 = _t16(nc, ps, ident, gvecs[0][:], 64)
    nc.scalar.copy(xcT0[0:64, :], t0)
    t1 = _t16(nc, ps, ident, gvecs[1][:], 64)
    nc.scalar.copy(xcT0[64:128, :], t1)
    xcT1 = sb.tile([128, GPC], f32, tag="xcT1")
    t2 = _t16(nc, ps, ident, xt_sb[:], 128)
    nc.scalar.copy(xcT1[:], t2)
    ones = sb.tile([128, GPC], f32, tag="ones")
    nc.vector.memset(ones[:], 0.0)
    nc.vector.memset(ones[0:1, :], 1.0)

    fc1Wk = sb.tile([128, 3, 128], f32, tag="fc1w")
    nc.sync.dma_start(fc1Wk[:], fc1WkD.ap().rearrange("k r n -> r k n"))
    xc1_ps_t = ps.tile([128, 1280], f32, tag="big")
    xc1_ps = xc1_ps_t[0:GPC, 0:128]
    for k, lt in enumerate((xcT0, xcT1, ones)):
        nc.tensor.matmul(xc1_ps[:], lt[:], fc1Wk[:, k, :], start=(k == 0), stop=(k == 2))
    xc1 = sb.tile([GPC, 128], f32, tag="xc1")
    nc.scalar.activation(xc1[:], xc1_ps[:],
                         mybir.ActivationFunctionType.Lrelu, alpha=0.01)
    xc1T = sb.tile([128, GPC], f32, tag="xc1T")
    t3 = _t16(nc, ps, ident, xc1[:], 128)
    nc.scalar.copy(xc1T[:], t3)

    fc2Wk = sb.tile([128, 2, 32], f32, tag="fc2w")
    nc.sync.dma_start(fc2Wk[:], fc2WkD.ap().rearrange("k r n -> r k n"))
    xc2_ps_t = ps.tile([128, 1280], f32, tag="big")
    xc2_ps = xc2_ps_t[0:GPC, 0:32]
    for k, lt in enumerate((xc1T, ones)):
        nc.tensor.matmul(xc2_ps[:], lt[:], fc2Wk[:, k, :], start=(k == 0), stop=(k == 1))
    xc2 = sb.tile([GPC, 32], f32, tag="xc2")
    nc.scalar.activation(xc2[:], xc2_ps[:],
                         mybir.ActivationFunctionType.Lrelu, alpha=0.01)
    xc2T = sb.tile([128, GPC], f32, tag="xc2T")
    nc.vector.memset(xc2T[:], 0.0)
    t4 = _t16(nc, ps, ident, xc2[:], 32)
    nc.scalar.copy(xc2T[0:32, :], t4)
    nc.vector.memset(xc2T[32:33, :], 1.0)

    outWk = sb.tile([128, 1], f32, tag="outw")
    nc.sync.dma_start(outWk[:], outWkD[:])
    out_ps_t = ps.tile([128, 1280], f32, tag="big")
    out_ps = out_ps_t[0:GPC, 0:1]
    nc.tensor.matmul(out_ps[:], xc2T[:], outWk[:], start=True, stop=True)
    outsb = sb.tile([GPC, 1], f32, tag="outsb")
    nc.vector.tensor_copy(outsb[:], out_ps[:])
    w = nc.sync.dma_start(out_loc[:], outsb[:])
    ag = nc.gpsimd.collective_compute(
        "AllGather", mybir.AluOpType.bypass,
        replica_groups=[list(range(NC))],
        ins=[out_loc[:]], outs=[out_glob[:]])
    add_dep_helper(ag.ins, w.ins, reason="AG out after write")
    fin = sb.tile([G, 1], f32, tag="fin")
    r = nc.sync.dma_start(fin[:], out_glob[:])
    add_dep_helper(r.ins, ag.ins, reason="read after AG out")
    nc.sync.dma_start(outD[:], fin[:])


def _build_program(st1, st2):
    nc = bacc.Bacc("TRN2", target_bir_lowering=False, debug=False, num_devices=NC)
    d = nc.dram_tensor
    identD = d("identity", [128, 128], f32, kind="ExternalInput")

    with tile.TileContext(nc) as tc:
        with (
            tc.tile_pool(name="base", bufs=1) as base,
            tc.tile_pool(name="ps", bufs=2, space="PSUM") as ps,
        ):
            lib = nc.gpsimd.load_library(library_config.mlp)
            ident = base.tile([128, 128], f32, tag="ident")
            nc.sync.dma_start(ident[:], identD[:])
            identb = base.tile([128, 128], bf16, tag="identb")
            nc.vector.tensor_copy(identb[:], ident[:])
            pools = {"ps": ps, "ident": ident, "identb": identb, "lib": lib,
                     "base": base}

            pool_sb = base.tile([128, 37, GPC], f32r, tag="pool_sb")
            nc.vector.memset(pool_sb[:].bitcast(f32), 0.0)
            nc.vector.memset(pool_sb[0:1, 36, :].bitcast(f32), 1.0)

            br1 = _decl_branch(nc, pools, "b1_", st1)
            br2 = _decl_branch(nc, pools, "b2_", st2)

            with nc.named_scope("tbuild"):
                with tc.tile_pool(name="tb1", bufs=2) as sb:
                    _build_tbuild(nc, pools, br1, sb)
                with tc.tile_pool(name="tb2", bufs=2) as sb:
                    _build_tbuild(nc, pools, br2, sb)
            _ag_t(nc, br1)
            _ag_t(nc, br2)

            with tc.tile_pool(name="xtp", bufs=2) as sb:
                with nc.named_scope("xt"):
                    xt_sb = _build_xt(nc, pools, sb)

            with nc.named_scope("gat1"):
                with tc.tile_pool(name="gat1", bufs=2) as sb:
                    _gat_weights(nc, br1, sb)
                    for b in range(st1["NBLK"]):
                        S = _gat_load(nc, pools, br1, sb, b)
                        _build_gat_block(nc, pools, br1, sb, b, S)
            # interleave: branch-2 GAT (DVE-heavy) with branch-1 GCN (PE/DMA)
            PREF = 6
            with nc.named_scope("mid"):
                with (
                    tc.tile_pool(name="gat2", bufs=2) as sbA,
                    tc.tile_pool(name="gcn1", bufs=2) as sbB,
                ):
                    _gat_weights(nc, br2, sbA)
                    _gcn_weights(nc, br1, sbB)
                    squeue = [_gat_load(nc, pools, br2, sbA, b)
                              for b in range(min(PREF, st2["NBLK"]))]
                    _ag_z(nc, br1)
                    NB2, NB1 = st2["NBLK"], st1["NBLK"]
                    ng = 0  # next gat2 block to emit

                    def emit_gat2():
                        nonlocal ng
                        if ng < NB2:
                            _build_gat_block(nc, pools, br2, sbA, ng, squeue.pop(0))
                            if ng + PREF < NB2:
                                squeue.append(_gat_load(nc, pools, br2, sbA, ng + PREF))
                            ng += 1

                    for b in range(NB1):
                        emit_gat2()
                        emit_gat2()
                        _build_gcn_block(nc, pools, br1, sbB, b)
                    while ng < NB2:
                        emit_gat2()
                    _ag_z(nc, br2)
                    _pool_finalize(nc, pools, br1, sbB, pool_sb, 0)

            with nc.named_scope("gcn2"):
                with (
                    tc.tile_pool(name="gcn2", bufs=2) as sb,
                    tc.tile_pool(name="tailp", bufs=2) as sbT,
                ):
                    _gcn_weights(nc, br2, sb)
                    with nc.named_scope("gvec1"):
                        g1 = _build_gvec(nc, pools, sbT, pool_sb, 0)
                    for b in range(st2["NBLK"]):
                        _build_gcn_block(nc, pools, br2, sb, b, zbufs=3, ybufs=2)
                    _pool_finalize(nc, pools, br2, sb, pool_sb, 18)

            with nc.named_scope("tail"):
                with tc.tile_pool(name="tail2", bufs=2) as sb:
                    g2 = _build_gvec(nc, pools, sb, pool_sb, 1, fbufs=2)
                    _build_tail(nc, pools, sb, xt_sb, (g1, g2))

    nc.compile()
    return nc


# ---------------------------------------------------------------- entry point
def kernel(**inputs) -> np.ndarray:
    st1, pc1, sh1 = _prep_branch(inputs["x1"], inputs["edge_index1"], inputs["batch1"],
                                 inputs["p1_gatW"], inputs["p1_att_src"],
                                 inputs["p1_att_dst"])
    st2, pc2, sh2 = _prep_branch(inputs["x2"], inputs["edge_index2"], inputs["batch2"],
                                 inputs["p2_gatW"], inputs["p2_att_src"],
                                 inputs["p2_att_dst"])
    gatWk1, gcnWk1 = _pack_branch_weights(inputs["p1_gatW"], inputs["p1_gatb"],
                                          inputs["p1_gcnW"], inputs["p1_gcnb"])
    gatWk2, gcnWk2 = _pack_branch_weights(inputs["p2_gatW"], inputs["p2_gatb"],
                                          inputs["p2_gcnW"], inputs["p2_gcnb"])
    tail = _pack_tail(inputs)

    key = (st1["MAXG"], tuple(st1["nsub"]), tuple(st1["nidx"]),
           st2["MAXG"], tuple(st2["nsub"]), tuple(st2["nidx"]))
    if key not in _PROG_CACHE:
        _PROG_CACHE[key] = _build_program(st1, st2)
    nc = _PROG_CACHE[key]

    target = tail["target"]
    in_maps = []
    for c in range(NC):
        m = {"identity": tail["identity"],
             "target16": np.ascontiguousarray(target[c * GPC:(c + 1) * GPC]),
             "fcxtWk": tail["fcxtWk"], "fc1Wk": tail["fc1Wk"],
             "fc2Wk": tail["fc2Wk"], "outWk": tail["outWk"],
             "p1_fcg1Wk": tail["p1_fcg1Wk"], "p1_fcg2Wk": tail["p1_fcg2Wk"],
             "p2_fcg1Wk": tail["p2_fcg1Wk"], "p2_fcg2Wk": tail["p2_fcg2Wk"]}
        for pfx, pc, sh, gatWk, gcnWk in (("b1_", pc1, sh1, gatWk1, gcnWk1),
                                          ("b2_", pc2, sh2, gatWk2, gcnWk2)):
            p = pc[c]
            m[pfx + "Wsd"] = sh["Wsd"]
            m[pfx + "gatWk"] = gatWk
            m[pfx + "gcnWk"] = gcnWk
            for k in ("isrc", "mask", "maskT", "xslot", "dvi", "poison", "rcnt"):
                m[pfx + k] = p[k]
        in_maps.append(m)

    trace = bool(int(os.environ.get("KERNEL_TRACE", "0")))
    kw = {}
    if trace:
        kw = dict(trace=True, trace_cores=[0])
    res = run_bass_kernel_spmd(nc, in_maps, list(range(NC)), **kw)
    global LAST_RESULTS
    LAST_RESULTS = res
    return np.asarray(res.results[0]["out"], dtype=np.float32)
